# revision 1
# baseline (speedup 1.0000x reference)
"""Trainium2 Bass kernel for nn_DiffMPC2 (100-step diagonal-QP SGD recursion).

The reference iterates  u <- u - LR*(2*q*u + p)  100 times, i.e. the affine
per-element map  u <- a*u + b  with  a = 1 - 0.02*q,  b = -0.01*p.  Closed
form:  u_100 = a^100 * u0 + b * S_100,  S_100 = sum_{k<100} a^k.

Per element (f32), engines in brackets:
    L   = Ln(1 - 0.02*q)                [ACT]
    P   = Exp(100*L)  = a^100           [ACT]
    G   = Ln(2*q)                       [ACT]
    R   = Exp(-G)     = 0.5/q           [ACT]
    Sq  = Square(sqrt(.6468)*q - .6155) [ACT]  (= .6468q^2 - .99q + .3788)
    St  = -Sq - .6212                   [DVE tensor_scalar]
        = -1 + .99q - .6468q^2            (Taylor of -0.01*S_100; exact for
                                           small q where P-1 cancels in f32)
    Sl  = (P - 1 - EPS)*R               [DVE scalar_tensor_tensor]
        = -0.01*S_100 - EPS*R             (exact unless q small; the -EPS*R
                                           shift pushes it below St wherever
                                           its f32 noise matters)
    S'  = max(St, Sl)                   [DVE tensor_tensor]
    u   = P*u0 + S'*p                   [DVE x3]

Sharding: pure data parallel, batch split across 8 cores.  Each core gets
131072 rows x 4 ctrl cols = 524288 elems laid out as [128, 4096] f32.
Only Q[:,12:], p[:,12:], u_init are touched (x_init is dead): 8 MB of HBM
traffic per core, which is the memory roofline for this problem.  The three
inputs are host-packed into one DRAM tensor ([q | p | u0] per partition)
so each chunk needs a single input DMA.

Written in raw bass (explicit per-engine programs + semaphores): the
container's walrus build only allows ONE sync-wait per compute instruction,
which the Tile scheduler's automatic sem assignment keeps exceeding.  With
raw bass every wait is its own instruction.  Pipelined over N_CHUNKS column
chunks: input DMAs are all issued up front (per-chunk tiles + per-DMA
semaphores), ACT runs one-plus chunks ahead of DVE via split a/p/b
semaphores (Sq, then P, then R ready), and stores trail DVE per chunk.  GPSIMD is intentionally
unused: it shares SBUF ports with the DVE and running elementwise ops
there stalls both engines.
"""

import sys

for _p in (
    "/root/.axon_site",
    "/root/.axon_site/_ro/trn_rl_repo",
    "/root/.axon_site/_ro/pypackages",
):
    if _p not in sys.path:
        sys.path.append(_p)

import numpy as np

from concourse import bass, mybir
from concourse.bass_utils import run_bass_kernel_spmd

N_CORES = 8
B = 1048576
S_DIM = 12
C_DIM = 4
PARTS = 128
F_TOTAL = (B // N_CORES) * C_DIM // PARTS  # 4096
# Small first/last chunks shrink pipeline fill and drain; middle chunks
# amortize per-instruction overhead.
CHUNKS = [256, 512, 768, 1024, 1152, 384]
assert sum(CHUNKS) == F_TOTAL
N_CHUNKS = len(CHUNKS)
OFFS = [sum(CHUNKS[:i]) for i in range(N_CHUNKS)]
F_MAX = max(CHUNKS)
NSLOT = 4  # ACT->DVE handoff buffering

SQ_SCALE = 0.8042387962341309  # sqrt(0.6468)
SQ_BIAS = -0.6154888272285461  # -0.99 / (2*sqrt(0.6468))
ST_BIAS = -0.6211734414100647  # -(1 - SQ_BIAS^2)
# LUT-branch downshift: Sl = (P-1-EPS)*R.  EPS exceeds the worst-case f32
# noise in P (the rounding of 1-0.02q amplifies x100 through the exponent,
# ~3e-6, plus LUT spline error), so wherever Sl is unreliable it lands
# strictly below the Taylor branch and  S' = max(St, Sl)  picks St.  St
# truncates an alternating series, hence St <= true S' everywhere.
EPS = 6e-6

_nc_cache = None


def _build_bass():
    f32 = mybir.dt.float32
    u8 = mybir.dt.uint8
    Alu = mybir.AluOpType
    Act = mybir.ActivationFunctionType

    nc = bass.Bass()

    # Register activation-bias constants (Bass only pre-registers 0/1).
    # Ordering vs the ACT reads is via s_const, cheaper than a full barrier.
    const_memsets = []
    for val in (SQ_BIAS,):
        t = nc.alloc_sbuf_tensor(f"const-f32-{val}", [128, 1], f32)
        const_memsets.append(nc.gpsimd.memset(t.ap(), val))
        nc.const_aps.aps[(f32, val)] = t.ap()

    # Packed input: per partition [q | p | u0], each F_TOTAL wide.
    xin = nc.declare_dram_parameter("xin", [PARTS, 3 * F_TOTAL], f32, isOutput=False)
    uo = nc.declare_dram_parameter("uo", [PARTS, F_TOTAL], f32, isOutput=True)
    xr = xin.ap().rearrange("p (j f) -> p j f", j=3)

    def sb(name, cols, dtype=f32):
        return nc.alloc_sbuf_tensor(name, [PARTS, cols], dtype).ap()

    # Input tiles: one slot per chunk -- no reuse, so every input DMA can be
    # issued immediately with no compute-gating.
    tx = [
        sb(f"tx{c}", 3 * CHUNKS[c]).rearrange("p (j f) -> p j f", j=3)
        for c in range(N_CHUNKS)
    ]
    tP = [sb(f"tP{s}", F_MAX) for s in range(NSLOT)]
    tR = [sb(f"tR{s}", F_MAX) for s in range(NSLOT)]
    tSq = [sb(f"tSq{s}", F_MAX) for s in range(NSLOT)]
    # Engine-local scratch (in-order reuse is safe).
    tL = sb("tL", F_MAX)
    tG = sb("tG", F_MAX)
    tSt = sb("tSt", F_MAX)
    tS = sb("tS", F_MAX)
    tMx = sb("tMx", F_MAX)
    tr2 = sb("tr2", F_MAX)
    tr1 = sb("tr1", F_MAX)
    tout = sb("tout", F_TOTAL)

    # Per-DMA semaphores, each waited at its final value (16).  A single
    # cumulative DMA sem is racy with many DMAs in flight: the 16 SDMA
    # engines complete their slices of different DMAs at different rates,
    # so an intermediate threshold can be reached by increments from LATER
    # transfers while an earlier one is still partially in flight.
    s_inq = [nc.alloc_semaphore(f"s_inq{c}") for c in range(N_CHUNKS)]
    s_inpu = [nc.alloc_semaphore(f"s_inpu{c}") for c in range(N_CHUNKS)]

    with (
        nc.Block() as block,
        nc.semaphore("s_const") as s_const,
        nc.semaphore("s_acta") as s_acta,
        nc.semaphore("s_actp") as s_actp,
        nc.semaphore("s_actb") as s_actb,
        nc.semaphore("s_dve") as s_dve,
        nc.semaphore("s_out") as s_out,
    ):
        for ms in const_memsets:
            ms.then_inc(s_const, 1)

        @block.sync
        def _(sp):
            # q slices feed ACT (head of the dependency chain); p+u0 are
            # only needed by the last three DVE ops of a chunk.  Keep the q
            # stream one chunk ahead of the pu stream.
            def dma_q(c):
                sl = slice(OFFS[c], OFFS[c] + CHUNKS[c])
                sp.dma_start(out=tx[c][:, 0, :], in_=xr[:, 0, sl]).then_inc(
                    s_inq[c], 16
                )

            def dma_pu(c):
                sl = slice(OFFS[c], OFFS[c] + CHUNKS[c])
                sp.dma_start(out=tx[c][:, 1:3, :], in_=xr[:, 1:3, sl]).then_inc(
                    s_inpu[c], 16
                )

            dma_q(0)
            dma_q(1)
            for c in range(N_CHUNKS):
                dma_pu(c)
                if c + 2 < N_CHUNKS:
                    dma_q(c + 2)
            for c in range(N_CHUNKS):
                sp.wait_ge(s_dve, c + 1)
                sp.dma_start(
                    out=uo.ap()[:, OFFS[c] : OFFS[c] + CHUNKS[c]],
                    in_=tout[:, OFFS[c] : OFFS[c] + CHUNKS[c]],
                ).then_inc(s_out, 16)
            sp.wait_ge(s_out, 16 * N_CHUNKS)

        @block.scalar
        def _(act):
            # Warm the natural_log_exp activation-table set (and pick up the
            # bias constants) while the first input DMA is in flight; the
            # ~1.3us table load would otherwise sit on the critical path.
            act.wait_ge(s_const, len(const_memsets))
            act.activation(tL[:, :1], tG[:, :1], Act.Ln, bias=1.0, scale=0.0)
            act.activation(tG[:, :1], tL[:, :1], Act.Square, bias=SQ_BIAS, scale=0.0)
            for c in range(N_CHUNKS):
                s = c % NSLOT
                w = CHUNKS[c]
                tq = tx[c][:, 0, :]
                act.wait_ge(s_inq[c], 16)
                if c >= NSLOT:
                    # tP/tR/tSq slot reuse: DVE chunk c-NSLOT must be done.
                    act.wait_ge(s_dve, c - NSLOT + 1)
                act.activation(
                    tSq[s][:, :w], tq, Act.Square, bias=SQ_BIAS, scale=SQ_SCALE
                ).then_inc(s_acta, 1)
                act.activation(tL[:, :w], tq, Act.Ln, bias=1.0, scale=-0.02)
                act.activation(
                    tP[s][:, :w], tL[:, :w], Act.Exp, bias=0.0, scale=100.0
                ).then_inc(s_actp, 1)
                act.activation(tG[:, :w], tq, Act.Ln, bias=0.0, scale=2.0)
                act.activation(
                    tR[s][:, :w], tG[:, :w], Act.Exp, bias=0.0, scale=-1.0
                ).then_inc(s_actb, 1)

        @block.vector
        def _(v):
            for c in range(N_CHUNKS):
                s = c % NSLOT
                w = CHUNKS[c]
                tp_ = tx[c][:, 1, :]
                tu = tx[c][:, 2, :]
                sl = slice(OFFS[c], OFFS[c] + w)
                v.wait_ge(s_acta, c + 1)
                # St = -Sq + ST_BIAS = -1 + 0.99q - 0.6468q^2
                v.tensor_scalar(
                    tSt[:, :w], tSq[s][:, :w], -1.0, ST_BIAS, Alu.mult, Alu.add
                )
                v.wait_ge(s_inpu[c], 16)
                v.wait_ge(s_actp, c + 1)
                v.tensor_mul(tr1[:, :w], tP[s][:, :w], tu)
                v.wait_ge(s_actb, c + 1)
                # Sl = (P - 1 - EPS) * R  = -0.01*S_100 - EPS*R
                v.scalar_tensor_tensor(
                    tS[:, :w], tP[s][:, :w], 1.0 + EPS, tR[s][:, :w],
                    Alu.subtract, Alu.mult,
                )
                v.tensor_tensor(tMx[:, :w], tS[:, :w], tSt[:, :w], Alu.max)
                v.tensor_mul(tr2[:, :w], tMx[:, :w], tp_)
                v.tensor_add(tout[:, sl], tr1[:, :w], tr2[:, :w]).then_inc(s_dve, 1)

    return nc


def _get_nc():
    global _nc_cache
    if _nc_cache is None:
        _nc_cache = _build_bass()
    return _nc_cache


def _prep_in_maps(Q, p, u_init):
    q_u = np.ascontiguousarray(Q[:, S_DIM:], dtype=np.float32).reshape(
        N_CORES, PARTS, F_TOTAL
    )
    p_u = np.ascontiguousarray(p[:, S_DIM:], dtype=np.float32).reshape(
        N_CORES, PARTS, F_TOTAL
    )
    u0 = np.ascontiguousarray(u_init, dtype=np.float32).reshape(
        N_CORES, PARTS, F_TOTAL
    )
    xin = np.concatenate([q_u, p_u, u0], axis=2)  # [8, 128, 3*F_TOTAL]
    return [{"xin": xin[c]} for c in range(N_CORES)]


def kernel(x_init, Q, p, u_init):
    assert Q.shape == (B, S_DIM + C_DIM) and u_init.shape == (B, C_DIM)
    nc = _get_nc()
    in_maps = _prep_in_maps(Q, p, u_init)
    res = run_bass_kernel_spmd(nc, in_maps, list(range(N_CORES)))
    out = np.stack([res.results[c]["uo"] for c in range(N_CORES)])
    return out.reshape(B, C_DIM)



# revision 3
# speedup vs baseline: 1.3239x; 1.3239x over previous
"""Trainium2 Bass kernel for nn_DiffMPC2 (100-step diagonal-QP SGD recursion).

The reference iterates  u <- u - LR*(2*q*u + p)  100 times, i.e. the affine
per-element map  u <- a*u + b  with  a = 1 - 0.02*q,  b = -0.01*p.  Closed
form:  u_100 = P*u0 + S'*p,  P = a^100,  S' = (P-1)/(2q) in [-1, -0.4337).

v2 design (vs the f32 baseline at 44.4us):
  * All HBM traffic is bf16 (norm tolerance 2e-2; this scheme measures ~4e-3
    end-to-end): per core 3 MB in + 1 MB out instead of 8 MB.  q is clamped
    to >= 1e-7 on the host so no inf/NaN paths exist (true S'(0) = -1 is
    recovered by the clamp anyway); u0 is pre-doubled on the host.
  * ACT (f32 internal, natural_log_exp set), 4 passes:
        L = Ln(1 - 0.02q),  P = Exp(100 L)   [f32 -- the P-1 cancellation]
        G = Ln(q),          R = Exp(-G)      [bf16 = 1/q]
  * DVE (bf16 2x/4x perf modes; scalar_tensor_tensor is 1x on this HW so it
    is used only where a fused op is net-cheaper):
        Pm1h = P*0.5 - (1+EPS)/2        tensor_scalar  f32->bf16   2x_2p
        m1   = (Pm1h + (1+EPS)/2)*2u0   stt            = P*u0 exactly (1x)
        Sl   = Pm1h * R                 tensor_tensor  = (P-1-EPS)/(2q)  2x
        Slc  = max(Sl, -1)              tensor_scalar  4x
        m2   = Slc * p                  tensor_tensor  2x
        out  = m1 + m2                  tensor_tensor  2x
    The max(-1) clamp replaces the baseline's Taylor/max branch: EPS biases
    Sl downward wherever the f32 noise of P-1 is amplified by 1/q, and the
    true S' always exceeds -1, so clamping recovers those elements; EPS
    cancels exactly in m1.
  * ACT busy ~= DVE busy ~= 14us, input DMA ~9us -- both engines saturated.

Sharding: pure data parallel, batch split across 8 cores; per core
131072 rows x 4 ctrl cols = 524288 elems as [128, 4096] bf16.  Inputs are
host-packed per partition as [q | p | 2*u0] so each chunk needs one q DMA
and one p/u0 DMA.  Raw bass (explicit per-engine programs + semaphores):
the walrus build allows one sync-wait per compute instruction.
"""

import sys

for _p in (
    "/root/.axon_site",
    "/root/.axon_site/_ro/trn_rl_repo",
    "/root/.axon_site/_ro/pypackages",
):
    if _p not in sys.path:
        sys.path.append(_p)

import numpy as np
import ml_dtypes

from concourse import bass, mybir
from concourse.bass_utils import run_bass_kernel_spmd

N_CORES = 8
B = 1048576
S_DIM = 12
C_DIM = 4
PARTS = 128
F_TOTAL = (B // N_CORES) * C_DIM // PARTS  # 4096
# Small first chunk shrinks pipeline fill; middle chunks amortize overhead.
CHUNKS = [384, 768, 1024, 1024, 896]
assert sum(CHUNKS) == F_TOTAL
N_CHUNKS = len(CHUNKS)
OFFS = [sum(CHUNKS[:i]) for i in range(N_CHUNKS)]
F_MAX = max(CHUNKS)
NSLOT = 4  # ACT->DVE handoff buffering

# Sl = (P-1-EPS)/(2q).  EPS exceeds the worst-case f32 noise in P (argument
# rounding of 1-0.02q amplified x100 through the exponent ~ 3e-6), so noisy
# small-q elements land strictly below -1 and the max(-1) clamp recovers
# them.  m1 adds the same constant back, cancelling EPS exactly.
EPS = 6e-6
HALF1E = float(np.float32((1.0 + EPS) * 0.5))
Q_MIN = 1e-7  # host-side clamp: keeps Ln finite (true S'(0) = -1 anyway)

_nc_cache = None


def _build_bass():
    f32 = mybir.dt.float32
    bf16 = mybir.dt.bfloat16
    Alu = mybir.AluOpType
    Act = mybir.ActivationFunctionType

    nc = bass.Bass()

    # Packed input: per partition [q | p | 2*u0], each F_TOTAL wide, bf16.
    xin = nc.declare_dram_parameter("xin", [PARTS, 3 * F_TOTAL], bf16, isOutput=False)
    uo = nc.declare_dram_parameter("uo", [PARTS, F_TOTAL], bf16, isOutput=True)
    xr = xin.ap().rearrange("p (j f) -> p j f", j=3)

    def sb(name, cols, dtype):
        return nc.alloc_sbuf_tensor(name, [PARTS, cols], dtype).ap()

    # Input tiles: one slot per chunk -- every input DMA issues immediately.
    tx = [
        sb(f"tx{c}", 3 * CHUNKS[c], bf16).rearrange("p (j f) -> p j f", j=3)
        for c in range(N_CHUNKS)
    ]
    tP = [sb(f"tP{s}", F_MAX, f32) for s in range(NSLOT)]
    tR = [sb(f"tR{s}", F_MAX, bf16) for s in range(NSLOT)]
    # Engine-local scratch (in-order reuse is safe).
    tL = sb("tL", F_MAX, f32)
    tG = sb("tG", F_MAX, f32)
    tPm1h = sb("tPm1h", F_MAX, bf16)
    tSl = sb("tSl", F_MAX, bf16)
    tSlc = sb("tSlc", F_MAX, bf16)
    tm1 = sb("tm1", F_MAX, bf16)
    tm2 = sb("tm2", F_MAX, bf16)
    tout = sb("tout", F_TOTAL, bf16)

    # Per-DMA semaphores, each waited at its final value (16): a cumulative
    # DMA sem is racy with many DMAs in flight (16 SDMA engines complete
    # slices of different DMAs at different rates).
    s_inq = [nc.alloc_semaphore(f"s_inq{c}") for c in range(N_CHUNKS)]
    s_inpu = [nc.alloc_semaphore(f"s_inpu{c}") for c in range(N_CHUNKS)]

    with (
        nc.Block() as block,
        nc.semaphore("s_actp") as s_actp,
        nc.semaphore("s_actr") as s_actr,
        nc.semaphore("s_dve") as s_dve,
        nc.semaphore("s_out") as s_out,
    ):

        @block.sync
        def _(sp):
            # q slices feed ACT (head of the chain); p+u0 feed only the
            # m1/m2 DVE ops.  Keep the q stream one chunk ahead.
            def dma_q(c):
                sl = slice(OFFS[c], OFFS[c] + CHUNKS[c])
                sp.dma_start(out=tx[c][:, 0, :], in_=xr[:, 0, sl]).then_inc(
                    s_inq[c], 16
                )

            def dma_pu(c):
                sl = slice(OFFS[c], OFFS[c] + CHUNKS[c])
                sp.dma_start(out=tx[c][:, 1:3, :], in_=xr[:, 1:3, sl]).then_inc(
                    s_inpu[c], 16
                )

            dma_q(0)
            dma_q(1)
            for c in range(N_CHUNKS):
                dma_pu(c)
                if c + 2 < N_CHUNKS:
                    dma_q(c + 2)
            for c in range(N_CHUNKS):
                sp.wait_ge(s_dve, c + 1)
                sp.dma_start(
                    out=uo.ap()[:, OFFS[c] : OFFS[c] + CHUNKS[c]],
                    in_=tout[:, OFFS[c] : OFFS[c] + CHUNKS[c]],
                ).then_inc(s_out, 16)
            sp.wait_ge(s_out, 16 * N_CHUNKS)

        @block.scalar
        def _(act):
            # Warm the natural_log_exp table set while the first input DMA
            # is in flight (~2.7us otherwise on the critical path).
            act.activation(tL[:, :1], tG[:, :1], Act.Ln, bias=1.0, scale=0.0)
            act.activation(tG[:, :1], tL[:, :1], Act.Exp, bias=0.0, scale=0.0)
            for c in range(N_CHUNKS):
                s = c % NSLOT
                w = CHUNKS[c]
                tq = tx[c][:, 0, :]
                act.wait_ge(s_inq[c], 16)
                if c >= NSLOT:
                    # tP/tR slot reuse: DVE chunk c-NSLOT must be done.
                    act.wait_ge(s_dve, c - NSLOT + 1)
                act.activation(tL[:, :w], tq, Act.Ln, bias=1.0, scale=-0.02)
                act.activation(
                    tP[s][:, :w], tL[:, :w], Act.Exp, bias=0.0, scale=100.0
                ).then_inc(s_actp, 1)
                act.activation(tG[:, :w], tq, Act.Ln, bias=0.0, scale=1.0)
                act.activation(
                    tR[s][:, :w], tG[:, :w], Act.Exp, bias=0.0, scale=-1.0
                ).then_inc(s_actr, 1)

        @block.vector
        def _(v):
            for c in range(N_CHUNKS):
                s = c % NSLOT
                w = CHUNKS[c]
                tp_ = tx[c][:, 1, :]
                tu = tx[c][:, 2, :]
                sl = slice(OFFS[c], OFFS[c] + w)
                v.wait_ge(s_actp, c + 1)
                # Pm1h = (P - 1 - EPS)/2, rounded to bf16 AFTER the f32
                # subtraction (keeps the P-1 cancellation exact).
                v.tensor_scalar(
                    tPm1h[:, :w], tP[s][:, :w], 0.5, -HALF1E, Alu.mult, Alu.add
                )
                # m1 = (Pm1h + (1+EPS)/2) * 2u0 = P*u0 (EPS cancels exactly).
                v.wait_ge(s_inpu[c], 16)
                v.scalar_tensor_tensor(
                    tm1[:, :w], tPm1h[:, :w], HALF1E, tu, Alu.add, Alu.mult
                )
                v.wait_ge(s_actr, c + 1)
                # Sl = Pm1h * (1/q) = (P-1-EPS)/(2q)
                v.tensor_tensor(tSl[:, :w], tPm1h[:, :w], tR[s][:, :w], Alu.mult)
                v.tensor_scalar(tSlc[:, :w], tSl[:, :w], -1.0, None, Alu.max)
                v.tensor_tensor(tm2[:, :w], tSlc[:, :w], tp_, Alu.mult)
                v.tensor_tensor(
                    tout[:, sl], tm1[:, :w], tm2[:, :w], Alu.add
                ).then_inc(s_dve, 1)

    return nc


def _get_nc():
    global _nc_cache
    if _nc_cache is None:
        _nc_cache = _build_bass()
    return _nc_cache


def _prep_in_maps(Q, p, u_init):
    bf = ml_dtypes.bfloat16
    q_u = (
        np.maximum(np.ascontiguousarray(Q[:, S_DIM:], dtype=np.float32), Q_MIN)
        .astype(bf)
        .reshape(N_CORES, PARTS, F_TOTAL)
    )
    p_u = (
        np.ascontiguousarray(p[:, S_DIM:], dtype=np.float32)
        .astype(bf)
        .reshape(N_CORES, PARTS, F_TOTAL)
    )
    u0 = (
        (2.0 * np.ascontiguousarray(u_init, dtype=np.float32))
        .astype(bf)
        .reshape(N_CORES, PARTS, F_TOTAL)
    )
    xin = np.concatenate([q_u, p_u, u0], axis=2)  # [8, 128, 3*F_TOTAL] bf16
    return [{"xin": xin[c]} for c in range(N_CORES)]


def kernel(x_init, Q, p, u_init):
    assert Q.shape == (B, S_DIM + C_DIM) and u_init.shape == (B, C_DIM)
    nc = _get_nc()
    in_maps = _prep_in_maps(Q, p, u_init)
    res = run_bass_kernel_spmd(nc, in_maps, list(range(N_CORES)))
    out = np.stack([np.asarray(res.results[c]["uo"]) for c in range(N_CORES)])
    return out.reshape(B, C_DIM).astype(np.float32)
